# revision 40
# baseline (speedup 1.0000x reference)
"""Trainium2 Bass kernel for nn_Baseline_SelfGCN (gnn_message_passing).

Data-parallel over batch: 8 NeuronCores x 8 images each.

Final design. All large tensors ship as bfloat16 (halves HBM traffic, the
bottleneck for this memory-regime problem). x_gcn is pre-transposed on the
host to (HW, C); segment pooling runs as one 72-wide seg matmul per
(image, half) against a block-diagonal one-hot, accumulating all images into
four persistent PSUM chunks. x_global streams 0-4 interleaved in phase A
(DVE GAP reduces hide under the stream) and 5-7 inside the W2 stream
(reduces fill the DVE window before BN2). W1/W2 stream as kt-pair DMAs
pacing L1/L2. The midchain folds the count scales into partition-scaled BD
copies (BDm/BDs), applies BN1+relu, then produces the layer-2 lhsT directly
via the transposed bmm y1T = x1_chunk^T @ BD (layer-2 associativity:
BD@(x1@W2) == (BD@x1)@W2), so L2's psl2 chunks are already pre-BN x2 and the
tail is just BN2+relu -> cat DMA + bf16 part-mean matmul + BN(gn) -> bnf DMA.
BN folds run in (128, 144)/(128, 16) layouts (full-partition vector work);
mean factors (1/9, 1/256) fold into the BN scales. Engine placement respects
the hardware rule that GPSIMD cannot touch PSUM: all PSUM-reading elementwise
ops sit on DVE/Act, SBUF-only adds/copies on Pool.

Self-contained: hardcodes shapes; host side only shards/transposes/casts.
"""

import numpy as np
import ml_dtypes

import concourse.bass as bass
import concourse.mybir as mybir
import concourse.tile as tile
from concourse.masks import make_identity

F32 = mybir.dt.float32
BF16 = mybir.dt.bfloat16
I32 = mybir.dt.int32
AF = mybir.ActivationFunctionType
OP = mybir.AluOpType

BL = 8          # images per core
C = 2048
HW = 256        # Hf*Wf
NPARTS = 9      # graph nodes (parts 1..9)
R = BL * NPARTS  # 72 rows = (image, part)
EPS = 1e-5
NCH = 4         # 2048 / 512 N-chunks
KT = 16         # 2048 / 128 K-tiles
PF = 144        # NPARTS*C / 128 fold-layout free size
OUTW = 3 * C + 2 * NPARTS * C  # 43008


def legalize_waits(nc, max_waits=1):
    """Split multi-wait instructions: this walrus build allows only one
    embedded sync-wait per instruction; hoist extras onto standalone
    InstEventSemaphore waits on the same engine."""
    cnt = 0
    for fn in nc.m.functions:
        for blk in fn.blocks:
            out = []
            for inst in blk.instructions:
                si = inst.sync_info
                if si is not None and si.on_wait and len(si.on_wait) > max_waits:
                    waits = list(si.on_wait)
                    for w in waits[:-max_waits]:
                        cnt += 1
                        wi = mybir.InstEventSemaphore(
                            name=f"wsplit{cnt}_{inst.name}", ins=[], outs=[],
                            sync_info=mybir.SyncInfo(on_wait=[w], on_update=[]))
                        wi.engine = inst.engine
                        nc.register_instruction(wi)
                        out.append(wi)
                    si.on_wait = waits[-max_waits:]
                    inst.sync_info = si
                out.append(inst)
            blk.instructions = out
    return cnt


def _raw(ap, dims):
    """Build an AP over the same tensor with explicit [stride, count] dims."""
    return bass.AP(tensor=ap.tensor, offset=ap.offset, ap=dims)


def build_bass():
    nc = bass.Bass()

    xt_p = nc.declare_dram_parameter("x_gcn_t", [BL, HW, C], BF16, isOutput=False)
    xg_p = nc.declare_dram_parameter("x_global", [BL, C, HW], BF16, isOutput=False)
    mk_p = nc.declare_dram_parameter("mask_p", [128, BL, 2], I32, isOutput=False)
    bd_p = nc.declare_dram_parameter("adj_bdt", [R, R], BF16, isOutput=False)
    w1_p = nc.declare_dram_parameter("W1", [C, C], BF16, isOutput=False)
    w2_p = nc.declare_dram_parameter("W2", [C, C], BF16, isOutput=False)
    b1_p = nc.declare_dram_parameter("b1", [C], F32, isOutput=False)
    b2_p = nc.declare_dram_parameter("b2", [C], F32, isOutput=False)
    bn1 = {k: nc.declare_dram_parameter(k + "1", [NPARTS * C], F32, isOutput=False)
           for k in ("g", "be", "rm", "rv")}
    bn2 = {k: nc.declare_dram_parameter(k + "2", [NPARTS * C], F32, isOutput=False)
           for k in ("g", "be", "rm", "rv")}
    gb = {k: nc.declare_dram_parameter("gb_" + k, [C], F32, isOutput=False)
          for k in ("g", "b", "rm", "rv")}
    gn = {k: nc.declare_dram_parameter("gn_" + k, [C], F32, isOutput=False)
          for k in ("g", "b", "rm", "rv")}
    out_p = nc.declare_dram_parameter("out", [BL, OUTW], F32, isOutput=True)

    with tile.TileContext(nc) as tc:
        with (
            tc.tile_pool(name="consts", bufs=1) as cs,
            tc.tile_pool(name="ps", bufs=8, space="PSUM") as ps,
            tc.tile_pool(name="dram", bufs=1, space="DRAM") as dp,
        ):
            # ---------------- persistent tiles ----------------
            BD = cs.tile([R, R], BF16)
            BDm = cs.tile([R, R], BF16)   # BD partition-scaled by 1/count
            BDs = cs.tile([R, R], BF16)   # BD partition-scaled by keep/count
            mr = cs.tile([128, BL, 2], I32)
            mrf = cs.tile([128, BL, 2], F32)
            # block-diagonal one-hot: oh[:, b, h, 9b:9b+9] is image b's
            # one-hot, other columns zero -> all-image seg pooling uses it
            # as a 72-wide stationary with per-(b,h) k-tiles
            oh = cs.tile([128, BL, 2, R], BF16)
            mfT = cs.tile([128, KT, R], BF16)
            G = cs.tile([128, BL, KT], BF16)
            Gn = cs.tile([128, BL, KT], F32)
            selfm98 = cs.tile([NPARTS, BL], F32)
            rec98 = cs.tile([NPARTS, BL], F32)
            mrec72 = cs.tile([R, 1], F32)
            srec72 = cs.tile([R, 1], F32)
            sgb = cs.tile([128, KT], F32)   # c = p*16 + j (matches G layout)
            tgb = cs.tile([128, KT], F32)
            sgn16 = cs.tile([128, KT], F32)
            tgn16 = cs.tile([128, KT], F32)
            sgn8 = cs.tile([BL, C], BF16)
            tgn8 = cs.tile([BL, C], BF16)
            # layer-1 and layer-2 BN reps share slots (srep2 loads after BN1)
            srep = {}
            for li in (1, 2):
                for k in ("s", "t"):
                    srep[(li, k)] = cs.tile([R, C], BF16, tag=f"rep_{k}",
                                            name=f"rep_{k}{li}")

            # DRAM scratch
            scr = {(li, k): dp.tile([NPARTS, C], BF16, name=f"scr_{k}{li}")
                   for li in (1, 2) for k in ("s", "t")}
            scr_b = {1: dp.tile([NPARTS, C], F32, name="scr_b1"),
                     2: dp.tile([NPARTS, C], F32, name="scr_b2")}
            scr_gn = {k: dp.tile([C], BF16, name=f"scr_gn{k}") for k in ("s", "t")}
            scr_sm = dp.tile([NPARTS, BL], F32, name="scr_sm")
            scr_rc = dp.tile([NPARTS, BL], F32, name="scr_rc")

            # mask first: unblocks onehot compute immediately
            nc.sync.dma_start(out=mr[:], in_=mk_p[:])

            # ---------------- constants (gpsimd, no DMA) ----------------
            ident = cs.tile([128, 128], BF16)
            make_identity(nc, ident[:])

            iota_i = cs.tile([128, NPARTS], I32)
            nc.gpsimd.iota(iota_i[:], pattern=[[1, NPARTS]], base=1, channel_multiplier=0)
            iota_f = cs.tile([128, NPARTS], F32)
            nc.gpsimd.tensor_copy(out=iota_f[:], in_=iota_i[:])

            ones_col = cs.tile([128, 1], BF16)
            nc.gpsimd.memset(ones_col[:], 1.0)

            # strictly-lower-triangular L9: L[q,p] = 1 if q < p
            L9 = cs.tile([NPARTS, NPARTS], BF16)
            nc.gpsimd.memset(L9[:], 0.0)
            nc.gpsimd.affine_select(
                out=L9[:], in_=L9[:], compare_op=OP.is_ge, fill=1.0,
                base=0, pattern=[[-1, NPARTS]], channel_multiplier=1,
            )

            # block "mean over parts" matrix (72, 8): ones on image blocks
            # (the 1/NPARTS is folded into the BN(gn) scale)
            onesblk = cs.tile([R, BL], BF16)
            nc.gpsimd.memset(onesblk[:], 1.0)
            nc.gpsimd.affine_select(
                out=onesblk[:], in_=onesblk[:], compare_op=OP.is_ge, fill=0.0,
                base=0, pattern=[[-NPARTS, BL]], channel_multiplier=1)
            nc.gpsimd.affine_select(
                out=onesblk[:], in_=onesblk[:], compare_op=OP.is_ge, fill=0.0,
                base=NPARTS - 1, pattern=[[NPARTS, BL]], channel_multiplier=-1)

            # one-hot compare ops (DVE, first in its queue)
            nc.gpsimd.memset(oh[:], 0.0)
            nc.vector.tensor_copy(out=mrf[:], in_=mr[:])
            for b in range(BL):
                for h in range(2):
                    nc.vector.tensor_scalar(
                        out=oh[:, b, h, NPARTS * b:NPARTS * (b + 1)],
                        in0=iota_f[:],
                        scalar1=mrf[:, b, h:h + 1], scalar2=None, op0=OP.is_equal)

            with (
                tc.tile_pool(name="small", bufs=2) as sp,
                tc.tile_pool(name="stream", bufs=3) as stream,
                tc.tile_pool(name="foldtmp", bufs=1) as ftp,
                tc.tile_pool(name="wp", bufs=4) as wp,
                tc.tile_pool(name="mm", bufs=1) as mm,
                tc.tile_pool(name="stage", bufs=3) as stg,
            ):
                # fold tiles (computed during phase A)
                fold = {}
                for li in (1, 2):
                    for k in ("st", "gt", "tt", "rmt", "bet", "bt"):
                        fold[(li, k)] = ftp.tile([128, PF], F32, tag=f"f_{k}",
                                                 name=f"f_{k}{li}")
                    for k in ("sb", "tb"):
                        fold[(li, k)] = ftp.tile([128, PF], BF16, tag=f"f_{k}",
                                                 name=f"f_{k}{li}")
                gf = {}
                for pr in ("gb", "gn"):
                    for k in ("g", "b", "rm", "rv"):
                        gf[(pr, k)] = ftp.tile([128, KT], F32, tag=f"gf_{k}",
                                               name=f"gf_{pr}{k}")
                gnb = {k: ftp.tile([128, KT], BF16, tag=f"gnb_{k}",
                                   name=f"gnb{k}") for k in ("s", "t")}

                def emit_small_dmas(step):
                    """Tiny loads/bounces interleaved between stream DMAs."""
                    if step == 0:
                        nc.sync.dma_start(out=BD[:], in_=bd_p[:])
                        nc.sync.dma_start(
                            out=scr_b[1][:],
                            in_=b1_p[None, :].to_broadcast([NPARTS, C]))
                        nc.sync.dma_start(
                            out=scr_b[2][:],
                            in_=b2_p[None, :].to_broadcast([NPARTS, C]))
                    elif step == 1:
                        for li, bnp in ((1, bn1), (2, bn2)):
                            for k, src in (("st", bnp["rv"]), ("gt", bnp["g"]),
                                           ("rmt", bnp["rm"]), ("bet", bnp["be"])):
                                nc.sync.dma_start(
                                    out=fold[(li, k)][:],
                                    in_=src.rearrange("(p f) -> p f", f=PF))
                            nc.sync.dma_start(
                                out=fold[(li, "bt")][:],
                                in_=_raw(scr_b[li][:], [[PF, 128], [1, PF]]))
                        for pr, d in (("gb", gb), ("gn", gn)):
                            for k in ("g", "b", "rm", "rv"):
                                nc.sync.dma_start(
                                    out=gf[(pr, k)][:],
                                    in_=d[k].rearrange("(p j) -> p j", j=KT))
                    elif step == 2:
                        # fold compute done by now; write scr
                        for li in (1, 2):
                            for k, t in (("s", "sb"), ("t", "tb")):
                                nc.sync.dma_start(
                                    out=_raw(scr[(li, k)][:],
                                             [[PF, 128], [1, PF]]),
                                    in_=fold[(li, t)][:])
                        nc.sync.dma_start(
                            out=scr_gn["s"][:].rearrange("(p j) -> p j", j=KT),
                            in_=gnb["s"][:])
                        nc.sync.dma_start(
                            out=scr_gn["t"][:].rearrange("(p j) -> p j", j=KT),
                            in_=gnb["t"][:])
                    elif step == 3:
                        nc.sync.dma_start(
                            out=sgn8[:],
                            in_=_raw(scr_gn["s"][:], [[0, BL], [1, C]]))
                        nc.sync.dma_start(
                            out=tgn8[:],
                            in_=_raw(scr_gn["t"][:], [[0, BL], [1, C]]))
                        # layer-1 BN reps (scr written at step 2)
                        for k in ("s", "t"):
                            nc.sync.dma_start(
                                out=srep[(1, k)][:],
                                in_=_raw(scr[(1, k)][:],
                                         [[0, BL], [C, NPARTS], [1, C]]))

                def emit_folds():
                    """BN fold arithmetic in (128, PF) / (128, KT) layouts."""
                    for li in (1, 2):
                        st, gt, tt = (fold[(li, "st")], fold[(li, "gt")],
                                      fold[(li, "tt")])
                        rmt, bet, bt = (fold[(li, "rmt")], fold[(li, "bet")],
                                        fold[(li, "bt")])
                        nc.vector.tensor_scalar_add(st[:], st[:], EPS)
                        nc.scalar.activation(out=st[:], in_=st[:], func=AF.Sqrt)
                        nc.vector.reciprocal(out=st[:], in_=st[:])
                        nc.vector.tensor_mul(st[:], st[:], gt[:])
                        nc.vector.tensor_sub(tt[:], bt[:], rmt[:])
                        nc.vector.tensor_mul(tt[:], tt[:], st[:])
                        nc.vector.tensor_add(tt[:], tt[:], bet[:])
                        nc.vector.tensor_copy(out=fold[(li, "sb")][:], in_=st[:])
                        nc.vector.tensor_copy(out=fold[(li, "tb")][:], in_=tt[:])
                    for pr, dst_s, dst_t, inv in (("gb", sgb, tgb, 1.0 / HW),
                                                  ("gn", sgn16, tgn16,
                                                   1.0 / NPARTS)):
                        nc.vector.tensor_scalar_add(dst_s[:], gf[(pr, "rv")][:],
                                                    EPS)
                        nc.scalar.activation(out=dst_s[:], in_=dst_s[:],
                                             func=AF.Sqrt)
                        nc.vector.reciprocal(out=dst_s[:], in_=dst_s[:])
                        nc.vector.tensor_mul(dst_s[:], dst_s[:],
                                             gf[(pr, "g")][:])
                        nc.vector.tensor_mul(dst_t[:], gf[(pr, "rm")][:],
                                             dst_s[:])
                        nc.vector.tensor_sub(dst_t[:], gf[(pr, "b")][:],
                                             dst_t[:])
                        # fold mean normalization into the scale
                        nc.vector.tensor_scalar_mul(dst_s[:], dst_s[:], inv)
                    nc.vector.tensor_copy(out=gnb["s"][:], in_=sgn16[:])
                    nc.vector.tensor_copy(out=gnb["t"][:], in_=tgn16[:])

                # ------------- phase A: x streams + pooling + GAP ----------
                # all-image seg pooling: mfeat (72, 2048) accumulates over
                # 16 (image, half) k-tiles with the block-diag one-hot
                psm = [ps.tile([R, 512], F32, tag="ps", name=f"psm{n}")
                       for n in range(NCH)]
                for b in range(BL):
                    xt = stream.tile([128, 2, C], BF16, tag="xt", name=f"xt{b}")
                    nc.sync.dma_start(
                        out=xt[:],
                        in_=xt_p[b].rearrange("(h p) c -> p h c", p=128))
                    if b < 5:
                        xg = stream.tile([128, KT, HW], BF16, tag="xg",
                                         name=f"xg{b}")
                        nc.sync.dma_start(
                            out=xg[:],
                            in_=xg_p[b].rearrange("(p j) hw -> p j hw", p=128))
                    if b == 0:
                        # loads first, then the fold arithmetic that uses them
                        emit_small_dmas(0)
                        emit_small_dmas(1)
                        emit_folds()
                    elif b in (2, 4):
                        emit_small_dmas(b // 2 + 1)
                    for h in range(2):
                        for n in range(NCH):
                            nc.tensor.matmul(
                                psm[n][:], oh[:, b, h, :],
                                xt[:, h, 512 * n:512 * (n + 1)],
                                start=(b == 0 and h == 0),
                                stop=(b == BL - 1 and h == 1))
                    if b < 5:
                        with nc.allow_low_precision(reason="GAP bf16"):
                            nc.vector.reduce_sum(out=G[:, b, :], in_=xg[:],
                                                 axis=mybir.AxisListType.X)

                # counts/drop logic (Pool + PE; runs behind the stream)
                for b in range(BL):
                    pcnt = ps.tile([NPARTS, 1], F32, tag="ps", name=f"pcnt{b}")
                    for h in range(2):
                        nc.tensor.matmul(pcnt[:],
                                         oh[:, b, h, NPARTS * b:NPARTS * (b + 1)],
                                         ones_col[:],
                                         start=(h == 0), stop=(h == 1))
                    nc.vector.tensor_scalar_add(rec98[:, b:b + 1], pcnt[:], 1e-8)
                    nc.vector.reciprocal(out=rec98[:, b:b + 1],
                                         in_=rec98[:, b:b + 1])
                    pres = sp.tile([NPARTS, 1], BF16, tag="pres", name=f"pres{b}")
                    nc.vector.tensor_scalar(out=pres[:], in0=pcnt[:], scalar1=0.0,
                                            scalar2=None, op0=OP.is_gt)
                    ppre = ps.tile([NPARTS, 1], F32, tag="ps", name=f"ppre{b}")
                    nc.tensor.matmul(ppre[:], L9[:], pres[:], start=True, stop=True)
                    isz = sp.tile([NPARTS, 1], F32, tag="isz", name=f"isz{b}")
                    nc.vector.tensor_scalar(out=isz[:], in0=ppre[:], scalar1=0.0,
                                            scalar2=None, op0=OP.is_equal)
                    nc.gpsimd.tensor_mul(isz[:], isz[:], pres[:])
                    nc.gpsimd.tensor_scalar(out=selfm98[:, b:b + 1], in0=isz[:],
                                            scalar1=-1.0, scalar2=1.0,
                                            op0=OP.mult, op1=OP.add)

                # mfeat (psum) -> SBUF bf16 -> transpose into mfT (grouped)
                mfsb = mm.tile([R, C], BF16, tag="mfsb")
                for n in range(NCH):
                    sl = slice(512 * n, 512 * (n + 1))
                    if n % 2 == 0:
                        nc.scalar.activation(out=mfsb[:, sl], in_=psm[n][:],
                                             func=AF.Copy)
                    else:
                        nc.vector.tensor_copy(out=mfsb[:, sl], in_=psm[n][:])
                for g4 in range(4):
                    pmt = ps.tile([128, 4, R], BF16, tag="ps", name=f"pmt{g4}")
                    for k in range(4):
                        kt = 4 * g4 + k
                        nc.tensor.transpose(pmt[:, k, :],
                                            mfsb[:, 128 * kt:128 * (kt + 1)],
                                            ident[0:R, 0:R])
                    eng = (nc.vector, nc.scalar, nc.vector, nc.scalar)[g4]
                    if eng is nc.scalar:
                        eng.activation(out=mfT[:, 4 * g4:4 * g4 + 4, :],
                                       in_=pmt[:], func=AF.Copy)
                    else:
                        eng.tensor_copy(out=mfT[:, 4 * g4:4 * g4 + 4, :],
                                        in_=pmt[:])

                # ---- L1 n-outer (W1 resident) + lagged per-chunk midchain ---
                s_raw = mm.tile([R, C], BF16, tag="sraw")
                x1 = {br: mm.tile([R, C], BF16, tag=f"x1{br}", name=f"x1{br}")
                      for br in ("m", "s")}
                y1T = {br: mm.tile([128, KT, R], BF16, tag=f"y1T{br}",
                                   name=f"y1T{br}")
                       for br in ("m", "s")}

                psl1 = [ps.tile([R, 512], F32, tag="ps", name=f"psl1_{n}")
                        for n in range(NCH)]
                for j in range(KT // 2):
                    w = wp.tile([128, 2, C], BF16, tag="w", name=f"w1_{j}")
                    nc.sync.dma_start(
                        out=w[:],
                        in_=w1_p[256 * j:256 * (j + 1), :].rearrange(
                            "(k p) c -> p k c", p=128))
                    if j == 1:
                        # counts done by now (Pool/PE ran behind the stream)
                        nc.sync.dma_start(out=scr_sm[:], in_=selfm98[:])
                        nc.sync.dma_start(out=scr_rc[:], in_=rec98[:])
                    if j == 3:
                        nc.sync.dma_start(
                            out=mrec72[:],
                            in_=_raw(scr_rc[:], [[1, BL], [BL, NPARTS], [0, 1]]))
                        nc.sync.dma_start(
                            out=srec72[:],
                            in_=_raw(scr_sm[:], [[1, BL], [BL, NPARTS], [0, 1]]))
                        nc.gpsimd.tensor_mul(srec72[:], srec72[:], mrec72[:])
                        # count scales fold into partition-scaled BDs
                        nc.gpsimd.tensor_scalar(
                            out=BDm[:], in0=BD[:], scalar1=mrec72[:, 0:1],
                            scalar2=None, op0=OP.mult)
                        nc.gpsimd.tensor_scalar(
                            out=BDs[:], in0=BD[:], scalar1=srec72[:, 0:1],
                            scalar2=None, op0=OP.mult)
                    for k in range(2):
                        kt = 2 * j + k
                        for n in range(NCH):
                            nc.tensor.matmul(psl1[n][:], mfT[:, kt, :],
                                             w[:, k, 512 * n:512 * (n + 1)],
                                             start=(kt == 0), stop=(kt == KT - 1))

                # stage-major midchain: copies, all po1, BN1+relu, y1T bmms
                for n in range(NCH):
                    sl = slice(512 * n, 512 * (n + 1))
                    cp = (nc.scalar, nc.vector, nc.scalar, nc.vector)[n]
                    if cp is nc.scalar:
                        cp.activation(out=s_raw[:, sl], in_=psl1[n][:],
                                      func=AF.Copy)
                    else:
                        cp.tensor_copy(out=s_raw[:, sl], in_=psl1[n][:])
                po1 = {}
                for br, bd in (("m", BDm), ("s", BDs)):
                    for n in range(NCH):
                        po = ps.tile([R, 512], F32, tag="ps", name=f"po1{br}{n}")
                        nc.tensor.matmul(po[:], bd[:],
                                         s_raw[:, 512 * n:512 * (n + 1)],
                                         start=True, stop=True)
                        po1[(br, n)] = po
                for br in ("m", "s"):
                    add_eng = nc.vector if br == "m" else nc.gpsimd
                    for n in range(NCH):
                        sl = slice(512 * n, 512 * (n + 1))
                        xs = stg.tile([R, 512], F32, tag="xo", name=f"xo1{br}{n}")
                        nc.vector.tensor_tensor(xs[:], po1[(br, n)][:],
                                                srep[(1, "s")][:, sl], OP.mult)
                        add_eng.tensor_tensor(xs[:], xs[:],
                                              srep[(1, "t")][:, sl], OP.add)
                        nc.scalar.activation(out=x1[br][:, sl], in_=xs[:],
                                             func=AF.Relu)
                # transposed bmm: y1T chunks = x1_chunk^T @ BD directly
                for br in ("m", "s"):
                    for n in range(NCH):
                        pyt = ps.tile([128, 4, R], F32, tag="ps",
                                      name=f"pyt{br}{n}")
                        for k in range(4):
                            kt = 4 * n + k
                            nc.tensor.matmul(pyt[:, k, :],
                                             x1[br][:, 128 * kt:128 * (kt + 1)],
                                             BD[:], start=True, stop=True)
                        if br == "m":
                            nc.vector.tensor_copy(
                                out=y1T[br][:, 4 * n:4 * n + 4, :], in_=pyt[:])
                        else:
                            nc.scalar.activation(
                                out=y1T[br][:, 4 * n:4 * n + 4, :], in_=pyt[:],
                                func=AF.Copy)

                # ---------------- L2 (kt-pair paced by W2 DMAs) ----------
                psl2 = {br: [ps.tile([R, 512], F32, tag="ps",
                                     name=f"psl2_{br}_{n}")
                             for n in range(NCH)] for br in ("m", "s")}
                xg_late = []
                for j in range(KT // 2):
                    w = wp.tile([128, 2, C], BF16, tag="w", name=f"w2_{j}")
                    nc.sync.dma_start(
                        out=w[:],
                        in_=w2_p[256 * j:256 * (j + 1), :].rearrange(
                            "(k p) c -> p k c", p=128))
                    if j >= 5:
                        b = j  # images 5, 6, 7
                        xg = stream.tile([128, KT, HW], BF16, tag="xg",
                                         name=f"xg{b}")
                        nc.sync.dma_start(
                            out=xg[:],
                            in_=xg_p[b].rearrange("(p j) hw -> p j hw", p=128))
                        xg_late.append(xg)
                    for k in range(2):
                        kt = 2 * j + k
                        for br in ("m", "s"):
                            for n in range(NCH):
                                nc.tensor.matmul(
                                    psl2[br][n][:], y1T[br][:, kt, :],
                                    w[:, k, 512 * n:512 * (n + 1)],
                                    start=(kt == 0), stop=(kt == KT - 1))

                # late reduces fill the DVE idle window before BN2
                for i, b in enumerate(range(5, BL)):
                    with nc.allow_low_precision(reason="GAP bf16"):
                        nc.vector.reduce_sum(out=G[:, b, :], in_=xg_late[i][:],
                                             axis=mybir.AxisListType.X)

                # layer-2 BN reps into the shared slots (after BN1 reads;
                # emitted after the W2 DMAs so they can't block the stream)
                for k in ("s", "t"):
                    nc.sync.dma_start(
                        out=srep[(2, k)][:],
                        in_=_raw(scr[(2, k)][:],
                                 [[0, BL], [C, NPARTS], [1, C]]))

                # ---------------- BN2 + relu + outputs --------------
                cat_off = {"m": 3 * C, "s": 3 * C + NPARTS * C}
                bnf_off = {"m": C, "s": 2 * C}
                bnf = {br: stg.tile([BL, C], F32, tag=f"bnf{br}", bufs=1,
                                    name=f"bnf{br}")
                       for br in ("m", "s")}
                x2bs = {}
                for br in ("m", "s"):
                    catv = out_p[:, cat_off[br]:cat_off[br] + NPARTS * C
                                 ].rearrange("b (q c) -> b q c", c=C)
                    for n in range(NCH):
                        sl = slice(512 * n, 512 * (n + 1))
                        xs = stg.tile([R, 512], F32, tag="xo", name=f"xo2{br}{n}")
                        add_eng = nc.vector if br == "m" else nc.gpsimd
                        nc.vector.tensor_tensor(xs[:], psl2[br][n][:],
                                                srep[(2, "s")][:, sl], OP.mult)
                        add_eng.tensor_tensor(xs[:], xs[:],
                                              srep[(2, "t")][:, sl], OP.add)
                        x2c = stg.tile([R, 512], F32, tag="x2c", bufs=6,
                                       name=f"x2c{br}{n}")
                        nc.scalar.activation(out=x2c[:], in_=xs[:], func=AF.Relu)
                        nc.sync.dma_start(out=catv[:, :, sl], in_=x2c[:])
                        x2b = stg.tile([R, 512], BF16, tag="x2b", bufs=3,
                                       name=f"x2b{br}{n}")
                        nc.scalar.activation(out=x2b[:], in_=xs[:], func=AF.Relu)
                        x2bs[(br, n)] = x2b
                for br in ("m", "s"):
                    for n in range(NCH):
                        sl = slice(512 * n, 512 * (n + 1))
                        pf = ps.tile([BL, 512], F32, tag="ps", name=f"pf{br}{n}")
                        nc.tensor.matmul(pf[:], onesblk[:], x2bs[(br, n)][:],
                                         start=True, stop=True)
                        badd = nc.vector if br == "m" else nc.gpsimd
                        nc.vector.tensor_tensor(bnf[br][:, sl], pf[:],
                                                sgn8[:, sl], OP.mult)
                        badd.tensor_tensor(bnf[br][:, sl], bnf[br][:, sl],
                                           tgn8[:, sl], OP.add)
                    boff = bnf_off[br]
                    nc.sync.dma_start(out=out_p[:, boff:boff + C],
                                      in_=bnf[br][:])

                # ---------------- GAP BN(gb) + out ----------------
                nc.vector.tensor_tensor(
                    Gn[:], G[:],
                    sgb[:, None, :].to_broadcast([128, BL, KT]), OP.mult)
                nc.vector.tensor_tensor(
                    Gn[:], Gn[:],
                    tgb[:, None, :].to_broadcast([128, BL, KT]), OP.add)
                nc.sync.dma_start(
                    out=out_p[:, 0:C].rearrange("b (p j) -> p b j", j=KT),
                    in_=Gn[:])

    legalize_waits(nc)
    return nc


_CACHE = {}


def kernel(_run_kwargs=None, **inputs):
    run_kwargs = _run_kwargs or {}
    if "nc" not in _CACHE:
        _CACHE["nc"] = build_bass()
    nc = _CACHE["nc"]

    B = inputs["x_global"].shape[0]
    n_cores = 8
    bl = B // n_cores
    bf16 = ml_dtypes.bfloat16

    rep_f32 = ["b1", "b2", "g1", "be1", "rm1", "rv1",
               "g2", "be2", "rm2", "rv2",
               "gb_g", "gb_b", "gb_rm", "gb_rv",
               "gn_g", "gn_b", "gn_rm", "gn_rv"]
    w1 = np.ascontiguousarray(inputs["W1"]).astype(bf16)
    w2 = np.ascontiguousarray(inputs["W2"]).astype(bf16)

    in_maps = []
    for c in range(n_cores):
        sl = slice(c * bl, (c + 1) * bl)
        xg = inputs["x_gcn"][sl].reshape(bl, C, HW)
        adj = inputs["adj"][sl]
        bdt = np.zeros((R, R), np.float32)
        for i in range(bl):
            bdt[NPARTS * i:NPARTS * (i + 1), NPARTS * i:NPARTS * (i + 1)] = adj[i].T
        # downsampled mask packed as (p, b, h) with hw = h*128 + p
        mds = inputs["mask"][sl, 0, ::16, ::16].reshape(bl, 2, 128)
        m = {
            "x_gcn_t": np.ascontiguousarray(
                xg.transpose(0, 2, 1)).astype(bf16),
            "x_global": np.ascontiguousarray(
                inputs["x_global"][sl]).reshape(bl, C, HW).astype(bf16),
            "mask_p": np.ascontiguousarray(
                mds.transpose(2, 0, 1)).astype(np.int32),
            "adj_bdt": bdt.astype(bf16),
            "W1": w1,
            "W2": w2,
        }
        for k in rep_f32:
            m[k] = np.ascontiguousarray(inputs[k]).astype(np.float32)
        in_maps.append(m)

    from concourse.bass_utils import run_bass_kernel_spmd
    res = run_bass_kernel_spmd(nc, in_maps, list(range(n_cores)), **run_kwargs)
    out = np.concatenate([res.results[c]["out"] for c in range(n_cores)], axis=0)
    _CACHE["last_results"] = res
    return out


# revision 47
# speedup vs baseline: 1.0369x; 1.0369x over previous
"""Trainium2 Bass kernel for nn_Baseline_SelfGCN (gnn_message_passing).

Data-parallel over batch: 8 NeuronCores x 8 images each.

Final design. All large tensors ship as bfloat16 (halves HBM traffic, the
bottleneck for this memory-regime problem). x_gcn is pre-transposed on the
host to (HW, C); segment pooling runs as one 72-wide seg matmul per
(image, half) against a block-diagonal one-hot, accumulating all images into
four persistent PSUM chunks. x_global streams 0-4 interleaved in phase A
(DVE GAP reduces hide under the stream) and 5-7 inside the W2 stream
(reduces fill the DVE window before BN2). W1/W2 stream as kt-pair DMAs
pacing L1/L2. The midchain folds the count scales into partition-scaled BD
copies (BDm/BDs), applies BN1+relu, then produces the layer-2 lhsT directly
via the transposed bmm y1T = x1_chunk^T @ BD (layer-2 associativity:
BD@(x1@W2) == (BD@x1)@W2), so L2's psl2 chunks are already pre-BN x2 and the
tail is just BN2+relu -> cat DMA + bf16 part-mean matmul + BN(gn) -> bnf DMA.
BN folds run in (128, 144)/(128, 16) layouts (full-partition vector work);
mean factors (1/9, 1/256) fold into the BN scales. Engine placement respects
the hardware rule that GPSIMD cannot touch PSUM: all PSUM-reading elementwise
ops sit on DVE/Act, SBUF-only adds/copies on Pool.

Self-contained: hardcodes shapes; host side only shards/transposes/casts.
"""

import numpy as np
import ml_dtypes

import concourse.bass as bass
import concourse.mybir as mybir
import concourse.tile as tile
from concourse.masks import make_identity

F32 = mybir.dt.float32
BF16 = mybir.dt.bfloat16
I32 = mybir.dt.int32
AF = mybir.ActivationFunctionType
OP = mybir.AluOpType

BL = 8          # images per core
C = 2048
HW = 256        # Hf*Wf
NPARTS = 9      # graph nodes (parts 1..9)
R = BL * NPARTS  # 72 rows = (image, part)
EPS = 1e-5
NCH = 4         # 2048 / 512 N-chunks
KT = 16         # 2048 / 128 K-tiles
PF = 144        # NPARTS*C / 128 fold-layout free size
OUTW = 3 * C + 2 * NPARTS * C  # 43008


def legalize_waits(nc, max_waits=1):
    """Split multi-wait instructions: this walrus build allows only one
    embedded sync-wait per instruction; hoist extras onto standalone
    InstEventSemaphore waits on the same engine."""
    cnt = 0
    for fn in nc.m.functions:
        for blk in fn.blocks:
            out = []
            for inst in blk.instructions:
                si = inst.sync_info
                if si is not None and si.on_wait and len(si.on_wait) > max_waits:
                    waits = list(si.on_wait)
                    for w in waits[:-max_waits]:
                        cnt += 1
                        wi = mybir.InstEventSemaphore(
                            name=f"wsplit{cnt}_{inst.name}", ins=[], outs=[],
                            sync_info=mybir.SyncInfo(on_wait=[w], on_update=[]))
                        wi.engine = inst.engine
                        nc.register_instruction(wi)
                        out.append(wi)
                    si.on_wait = waits[-max_waits:]
                    inst.sync_info = si
                out.append(inst)
            blk.instructions = out
    return cnt


def _raw(ap, dims):
    """Build an AP over the same tensor with explicit [stride, count] dims."""
    return bass.AP(tensor=ap.tensor, offset=ap.offset, ap=dims)


def build_bass():
    nc = bass.Bass()

    xt_p = nc.declare_dram_parameter("x_gcn_t", [BL, HW, C], BF16, isOutput=False)
    xg_p = nc.declare_dram_parameter("x_global", [BL, C, HW], BF16, isOutput=False)
    mk_p = nc.declare_dram_parameter("mask_p", [128, BL, 2], I32, isOutput=False)
    bd_p = nc.declare_dram_parameter("adj_bdt", [R, R], BF16, isOutput=False)
    w1_p = nc.declare_dram_parameter("W1", [C, C], BF16, isOutput=False)
    w2_p = nc.declare_dram_parameter("W2", [C, C], BF16, isOutput=False)
    b1_p = nc.declare_dram_parameter("b1", [C], F32, isOutput=False)
    b2_p = nc.declare_dram_parameter("b2", [C], F32, isOutput=False)
    bn1 = {k: nc.declare_dram_parameter(k + "1", [NPARTS * C], F32, isOutput=False)
           for k in ("g", "be", "rm", "rv")}
    bn2 = {k: nc.declare_dram_parameter(k + "2", [NPARTS * C], F32, isOutput=False)
           for k in ("g", "be", "rm", "rv")}
    gb = {k: nc.declare_dram_parameter("gb_" + k, [C], F32, isOutput=False)
          for k in ("g", "b", "rm", "rv")}
    gn = {k: nc.declare_dram_parameter("gn_" + k, [C], F32, isOutput=False)
          for k in ("g", "b", "rm", "rv")}
    out_p = nc.declare_dram_parameter("out", [BL, OUTW], F32, isOutput=True)

    with tile.TileContext(nc) as tc:
        with (
            tc.tile_pool(name="consts", bufs=1) as cs,
            tc.tile_pool(name="ps", bufs=8, space="PSUM") as ps,
            tc.tile_pool(name="dram", bufs=1, space="DRAM") as dp,
        ):
            # ---------------- persistent tiles ----------------
            BD = cs.tile([R, R], BF16)
            BDm = cs.tile([R, R], BF16)   # BD partition-scaled by 1/count
            BDs = cs.tile([R, R], BF16)   # BD partition-scaled by keep/count
            mr = cs.tile([128, BL, 2], I32)
            mrf = cs.tile([128, BL, 2], F32)
            # block-diagonal one-hot: oh[:, b, h, 9b:9b+9] is image b's
            # one-hot, other columns zero -> all-image seg pooling uses it
            # as a 72-wide stationary with per-(b,h) k-tiles
            oh = cs.tile([128, BL, 2, R], BF16)
            mfT = cs.tile([128, KT, R], BF16)
            G = cs.tile([128, BL, KT], BF16)
            Gn = cs.tile([128, BL, KT], F32)
            selfm98 = cs.tile([NPARTS, BL], F32)
            rec98 = cs.tile([NPARTS, BL], F32)
            mrec72 = cs.tile([R, 1], F32)
            srec72 = cs.tile([R, 1], F32)
            sgb = cs.tile([128, KT], F32)   # c = p*16 + j (matches G layout)
            tgb = cs.tile([128, KT], F32)
            sgn16 = cs.tile([128, KT], F32)
            tgn16 = cs.tile([128, KT], F32)
            sgn8 = cs.tile([BL, C], BF16)
            tgn8 = cs.tile([BL, C], BF16)
            # layer-1 and layer-2 BN reps share slots (srep2 loads after BN1)
            srep = {}
            for li in (1, 2):
                for k in ("s", "t"):
                    srep[(li, k)] = cs.tile([R, C], BF16, tag=f"rep_{k}",
                                            name=f"rep_{k}{li}")

            # DRAM scratch
            scr = {(li, k): dp.tile([NPARTS, C], BF16, name=f"scr_{k}{li}")
                   for li in (1, 2) for k in ("s", "t")}
            scr_b = {1: dp.tile([NPARTS, C], F32, name="scr_b1"),
                     2: dp.tile([NPARTS, C], F32, name="scr_b2")}
            scr_gn = {k: dp.tile([C], BF16, name=f"scr_gn{k}") for k in ("s", "t")}
            scr_sm = dp.tile([NPARTS, BL], F32, name="scr_sm")
            scr_rc = dp.tile([NPARTS, BL], F32, name="scr_rc")

            # mask first: unblocks onehot compute immediately
            nc.sync.dma_start(out=mr[:], in_=mk_p[:])

            # ---------------- constants (gpsimd, no DMA) ----------------
            ident = cs.tile([128, 128], BF16)
            make_identity(nc, ident[:])

            iota_i = cs.tile([128, NPARTS], I32)
            nc.gpsimd.iota(iota_i[:], pattern=[[1, NPARTS]], base=1, channel_multiplier=0)
            iota_f = cs.tile([128, NPARTS], F32)
            nc.gpsimd.tensor_copy(out=iota_f[:], in_=iota_i[:])

            ones_col = cs.tile([128, 1], BF16)
            nc.gpsimd.memset(ones_col[:], 1.0)

            # strictly-lower-triangular L9: L[q,p] = 1 if q < p
            L9 = cs.tile([NPARTS, NPARTS], BF16)
            nc.gpsimd.memset(L9[:], 0.0)
            nc.gpsimd.affine_select(
                out=L9[:], in_=L9[:], compare_op=OP.is_ge, fill=1.0,
                base=0, pattern=[[-1, NPARTS]], channel_multiplier=1,
            )

            # block "mean over parts" matrix (72, 8): ones on image blocks
            # (the 1/NPARTS is folded into the BN(gn) scale)
            onesblk = cs.tile([R, BL], BF16)
            nc.gpsimd.memset(onesblk[:], 1.0)
            nc.gpsimd.affine_select(
                out=onesblk[:], in_=onesblk[:], compare_op=OP.is_ge, fill=0.0,
                base=0, pattern=[[-NPARTS, BL]], channel_multiplier=1)
            nc.gpsimd.affine_select(
                out=onesblk[:], in_=onesblk[:], compare_op=OP.is_ge, fill=0.0,
                base=NPARTS - 1, pattern=[[NPARTS, BL]], channel_multiplier=-1)

            # one-hot compare ops (DVE, first in its queue)
            nc.gpsimd.memset(oh[:], 0.0)
            nc.vector.tensor_copy(out=mrf[:], in_=mr[:])
            for b in range(BL):
                for h in range(2):
                    nc.vector.tensor_scalar(
                        out=oh[:, b, h, NPARTS * b:NPARTS * (b + 1)],
                        in0=iota_f[:],
                        scalar1=mrf[:, b, h:h + 1], scalar2=None, op0=OP.is_equal)

            with (
                tc.tile_pool(name="small", bufs=2) as sp,
                tc.tile_pool(name="stream", bufs=3) as stream,
                tc.tile_pool(name="foldtmp", bufs=1) as ftp,
                tc.tile_pool(name="wp", bufs=4) as wp,
                tc.tile_pool(name="mm", bufs=1) as mm,
                tc.tile_pool(name="stage", bufs=3) as stg,
            ):
                # fold tiles (computed during phase A)
                fold = {}
                for li in (1, 2):
                    for k in ("st", "gt", "tt", "rmt", "bet", "bt"):
                        fold[(li, k)] = ftp.tile([128, PF], F32, tag=f"f_{k}",
                                                 name=f"f_{k}{li}")
                    for k in ("sb", "tb"):
                        fold[(li, k)] = ftp.tile([128, PF], BF16, tag=f"f_{k}",
                                                 name=f"f_{k}{li}")
                gf = {}
                for pr in ("gb", "gn"):
                    for k in ("g", "b", "rm", "rv"):
                        gf[(pr, k)] = ftp.tile([128, KT], F32, tag=f"gf_{k}",
                                               name=f"gf_{pr}{k}")
                gnb = {k: ftp.tile([128, KT], BF16, tag=f"gnb_{k}",
                                   name=f"gnb{k}") for k in ("s", "t")}

                def emit_small_dmas(step):
                    """Tiny loads/bounces interleaved between stream DMAs."""
                    if step == 0:
                        nc.sync.dma_start(out=BD[:], in_=bd_p[:])
                        nc.sync.dma_start(
                            out=scr_b[1][:],
                            in_=b1_p[None, :].to_broadcast([NPARTS, C]))
                        nc.sync.dma_start(
                            out=scr_b[2][:],
                            in_=b2_p[None, :].to_broadcast([NPARTS, C]))
                    elif step == 1:
                        for li, bnp in ((1, bn1), (2, bn2)):
                            for k, src in (("st", bnp["rv"]), ("gt", bnp["g"]),
                                           ("rmt", bnp["rm"]), ("bet", bnp["be"])):
                                nc.sync.dma_start(
                                    out=fold[(li, k)][:],
                                    in_=src.rearrange("(p f) -> p f", f=PF))
                            nc.sync.dma_start(
                                out=fold[(li, "bt")][:],
                                in_=_raw(scr_b[li][:], [[PF, 128], [1, PF]]))
                        for pr, d in (("gb", gb), ("gn", gn)):
                            for k in ("g", "b", "rm", "rv"):
                                nc.sync.dma_start(
                                    out=gf[(pr, k)][:],
                                    in_=d[k].rearrange("(p j) -> p j", j=KT))
                    elif step == 2:
                        # fold compute done by now; write scr
                        for li in (1, 2):
                            for k, t in (("s", "sb"), ("t", "tb")):
                                nc.sync.dma_start(
                                    out=_raw(scr[(li, k)][:],
                                             [[PF, 128], [1, PF]]),
                                    in_=fold[(li, t)][:])
                        nc.sync.dma_start(
                            out=scr_gn["s"][:].rearrange("(p j) -> p j", j=KT),
                            in_=gnb["s"][:])
                        nc.sync.dma_start(
                            out=scr_gn["t"][:].rearrange("(p j) -> p j", j=KT),
                            in_=gnb["t"][:])
                    elif step == 3:
                        nc.sync.dma_start(
                            out=sgn8[:],
                            in_=_raw(scr_gn["s"][:], [[0, BL], [1, C]]))
                        nc.sync.dma_start(
                            out=tgn8[:],
                            in_=_raw(scr_gn["t"][:], [[0, BL], [1, C]]))
                        # layer-1 BN reps (scr written at step 2)
                        for k in ("s", "t"):
                            nc.sync.dma_start(
                                out=srep[(1, k)][:],
                                in_=_raw(scr[(1, k)][:],
                                         [[0, BL], [C, NPARTS], [1, C]]))

                def emit_folds():
                    """BN fold arithmetic in (128, PF) / (128, KT) layouts."""
                    for li in (1, 2):
                        st, gt, tt = (fold[(li, "st")], fold[(li, "gt")],
                                      fold[(li, "tt")])
                        rmt, bet, bt = (fold[(li, "rmt")], fold[(li, "bet")],
                                        fold[(li, "bt")])
                        nc.vector.tensor_scalar_add(st[:], st[:], EPS)
                        nc.scalar.activation(out=st[:], in_=st[:], func=AF.Sqrt)
                        nc.vector.reciprocal(out=st[:], in_=st[:])
                        nc.vector.tensor_mul(st[:], st[:], gt[:])
                        nc.vector.tensor_sub(tt[:], bt[:], rmt[:])
                        nc.vector.tensor_mul(tt[:], tt[:], st[:])
                        nc.vector.tensor_add(tt[:], tt[:], bet[:])
                        nc.vector.tensor_copy(out=fold[(li, "sb")][:], in_=st[:])
                        nc.vector.tensor_copy(out=fold[(li, "tb")][:], in_=tt[:])
                    for pr, dst_s, dst_t, inv in (("gb", sgb, tgb, 1.0 / HW),
                                                  ("gn", sgn16, tgn16,
                                                   1.0 / NPARTS)):
                        nc.vector.tensor_scalar_add(dst_s[:], gf[(pr, "rv")][:],
                                                    EPS)
                        nc.scalar.activation(out=dst_s[:], in_=dst_s[:],
                                             func=AF.Sqrt)
                        nc.vector.reciprocal(out=dst_s[:], in_=dst_s[:])
                        nc.vector.tensor_mul(dst_s[:], dst_s[:],
                                             gf[(pr, "g")][:])
                        nc.vector.tensor_mul(dst_t[:], gf[(pr, "rm")][:],
                                             dst_s[:])
                        nc.vector.tensor_sub(dst_t[:], gf[(pr, "b")][:],
                                             dst_t[:])
                        # fold mean normalization into the scale
                        nc.vector.tensor_scalar_mul(dst_s[:], dst_s[:], inv)
                    nc.vector.tensor_copy(out=gnb["s"][:], in_=sgn16[:])
                    nc.vector.tensor_copy(out=gnb["t"][:], in_=tgn16[:])

                # ------------- phase A: x streams + pooling + GAP ----------
                # all-image seg pooling: mfeat (72, 2048) accumulates over
                # 16 (image, half) k-tiles with the block-diag one-hot
                psm = [ps.tile([R, 512], F32, tag="ps", name=f"psm{n}")
                       for n in range(NCH)]
                for b in range(BL):
                    xt = stream.tile([128, 2, C], BF16, tag="xt", name=f"xt{b}")
                    nc.sync.dma_start(
                        out=xt[:],
                        in_=xt_p[b].rearrange("(h p) c -> p h c", p=128))
                    if b < 5:
                        xg = stream.tile([128, KT, HW], BF16, tag="xg",
                                         name=f"xg{b}")
                        nc.sync.dma_start(
                            out=xg[:],
                            in_=xg_p[b].rearrange("(p j) hw -> p j hw", p=128))
                    if b == 0:
                        # loads first, then the fold arithmetic that uses them
                        emit_small_dmas(0)
                        emit_small_dmas(1)
                        emit_folds()
                    elif b in (2, 4):
                        emit_small_dmas(b // 2 + 1)
                    for h in range(2):
                        for n in range(NCH):
                            nc.tensor.matmul(
                                psm[n][:], oh[:, b, h, :],
                                xt[:, h, 512 * n:512 * (n + 1)],
                                start=(b == 0 and h == 0),
                                stop=(b == BL - 1 and h == 1))
                    if b < 5:
                        with nc.allow_low_precision(reason="GAP bf16"):
                            nc.vector.reduce_sum(out=G[:, b, :], in_=xg[:],
                                                 axis=mybir.AxisListType.X)

                # counts/drop logic (Pool + PE; runs behind the stream)
                for b in range(BL):
                    pcnt = ps.tile([NPARTS, 1], F32, tag="ps", name=f"pcnt{b}")
                    for h in range(2):
                        nc.tensor.matmul(pcnt[:],
                                         oh[:, b, h, NPARTS * b:NPARTS * (b + 1)],
                                         ones_col[:],
                                         start=(h == 0), stop=(h == 1))
                    nc.vector.tensor_scalar_add(rec98[:, b:b + 1], pcnt[:], 1e-8)
                    nc.vector.reciprocal(out=rec98[:, b:b + 1],
                                         in_=rec98[:, b:b + 1])
                    pres = sp.tile([NPARTS, 1], BF16, tag="pres", name=f"pres{b}")
                    nc.vector.tensor_scalar(out=pres[:], in0=pcnt[:], scalar1=0.0,
                                            scalar2=None, op0=OP.is_gt)
                    ppre = ps.tile([NPARTS, 1], F32, tag="ps", name=f"ppre{b}")
                    nc.tensor.matmul(ppre[:], L9[:], pres[:], start=True, stop=True)
                    isz = sp.tile([NPARTS, 1], F32, tag="isz", name=f"isz{b}")
                    nc.vector.tensor_scalar(out=isz[:], in0=ppre[:], scalar1=0.0,
                                            scalar2=None, op0=OP.is_equal)
                    nc.gpsimd.tensor_mul(isz[:], isz[:], pres[:])
                    nc.gpsimd.tensor_scalar(out=selfm98[:, b:b + 1], in0=isz[:],
                                            scalar1=-1.0, scalar2=1.0,
                                            op0=OP.mult, op1=OP.add)

                # mfeat (psum) -> SBUF bf16 -> transpose into mfT (grouped)
                mfsb = mm.tile([R, C], BF16, tag="mfsb")
                for n in range(NCH):
                    sl = slice(512 * n, 512 * (n + 1))
                    if n % 2 == 0:
                        nc.scalar.activation(out=mfsb[:, sl], in_=psm[n][:],
                                             func=AF.Copy)
                    else:
                        nc.vector.tensor_copy(out=mfsb[:, sl], in_=psm[n][:])
                for g4 in range(4):
                    pmt = ps.tile([128, 4, R], BF16, tag="ps", name=f"pmt{g4}")
                    for k in range(4):
                        kt = 4 * g4 + k
                        nc.tensor.transpose(pmt[:, k, :],
                                            mfsb[:, 128 * kt:128 * (kt + 1)],
                                            ident[0:R, 0:R])
                    eng = (nc.vector, nc.scalar, nc.vector, nc.scalar)[g4]
                    if eng is nc.scalar:
                        eng.activation(out=mfT[:, 4 * g4:4 * g4 + 4, :],
                                       in_=pmt[:], func=AF.Copy)
                    else:
                        eng.tensor_copy(out=mfT[:, 4 * g4:4 * g4 + 4, :],
                                        in_=pmt[:])

                # ---- L1 n-outer (W1 resident) + lagged per-chunk midchain ---
                s_raw = mm.tile([R, C], BF16, tag="sraw")
                x1 = {br: mm.tile([R, C], BF16, tag=f"x1{br}", name=f"x1{br}")
                      for br in ("m", "s")}
                y1T = {br: mm.tile([128, KT, R], BF16, tag=f"y1T{br}",
                                   name=f"y1T{br}")
                       for br in ("m", "s")}

                psl1 = [ps.tile([R, 512], F32, tag="ps", name=f"psl1_{n}")
                        for n in range(NCH)]
                for j in range(KT // 2):
                    w = wp.tile([128, 2, C], BF16, tag="w", name=f"w1_{j}")
                    nc.sync.dma_start(
                        out=w[:],
                        in_=w1_p[256 * j:256 * (j + 1), :].rearrange(
                            "(k p) c -> p k c", p=128))
                    if j == 1:
                        # counts done by now (Pool/PE ran behind the stream)
                        nc.sync.dma_start(out=scr_sm[:], in_=selfm98[:])
                        nc.sync.dma_start(out=scr_rc[:], in_=rec98[:])
                    if j == 3:
                        nc.sync.dma_start(
                            out=mrec72[:],
                            in_=_raw(scr_rc[:], [[1, BL], [BL, NPARTS], [0, 1]]))
                        nc.sync.dma_start(
                            out=srec72[:],
                            in_=_raw(scr_sm[:], [[1, BL], [BL, NPARTS], [0, 1]]))
                        nc.gpsimd.tensor_mul(srec72[:], srec72[:], mrec72[:])
                        # count scales fold into partition-scaled BDs
                        nc.gpsimd.tensor_scalar(
                            out=BDm[:], in0=BD[:], scalar1=mrec72[:, 0:1],
                            scalar2=None, op0=OP.mult)
                        nc.gpsimd.tensor_scalar(
                            out=BDs[:], in0=BD[:], scalar1=srec72[:, 0:1],
                            scalar2=None, op0=OP.mult)
                    for k in range(2):
                        kt = 2 * j + k
                        for n in range(NCH):
                            nc.tensor.matmul(psl1[n][:], mfT[:, kt, :],
                                             w[:, k, 512 * n:512 * (n + 1)],
                                             start=(kt == 0), stop=(kt == KT - 1))

                # stage-major midchain: copies, all po1, BN1+relu, y1T bmms
                for n in range(NCH):
                    sl = slice(512 * n, 512 * (n + 1))
                    cp = (nc.scalar, nc.vector, nc.scalar, nc.vector)[n]
                    if cp is nc.scalar:
                        cp.activation(out=s_raw[:, sl], in_=psl1[n][:],
                                      func=AF.Copy)
                    else:
                        cp.tensor_copy(out=s_raw[:, sl], in_=psl1[n][:])
                po1 = {}
                for br, bd in (("m", BDm), ("s", BDs)):
                    for n in range(NCH):
                        po = ps.tile([R, 512], F32, tag="ps", name=f"po1{br}{n}")
                        nc.tensor.matmul(po[:], bd[:],
                                         s_raw[:, 512 * n:512 * (n + 1)],
                                         start=True, stop=True)
                        po1[(br, n)] = po
                for br in ("m", "s"):
                    add_eng = nc.vector if br == "m" else nc.gpsimd
                    for n in range(NCH):
                        sl = slice(512 * n, 512 * (n + 1))
                        xs = stg.tile([R, 512], F32, tag="xo", name=f"xo1{br}{n}")
                        nc.vector.tensor_tensor(xs[:], po1[(br, n)][:],
                                                srep[(1, "s")][:, sl], OP.mult)
                        add_eng.tensor_tensor(xs[:], xs[:],
                                              srep[(1, "t")][:, sl], OP.add)
                        nc.scalar.activation(out=x1[br][:, sl], in_=xs[:],
                                             func=AF.Relu)
                # transposed bmm: y1T chunks = x1_chunk^T @ BD directly
                for br in ("m", "s"):
                    for n in range(NCH):
                        pyt = ps.tile([128, 4, R], F32, tag="ps",
                                      name=f"pyt{br}{n}")
                        for k in range(4):
                            kt = 4 * n + k
                            nc.tensor.matmul(pyt[:, k, :],
                                             x1[br][:, 128 * kt:128 * (kt + 1)],
                                             BD[:], start=True, stop=True)
                        if br == "m":
                            nc.vector.tensor_copy(
                                out=y1T[br][:, 4 * n:4 * n + 4, :], in_=pyt[:])
                        else:
                            nc.scalar.activation(
                                out=y1T[br][:, 4 * n:4 * n + 4, :], in_=pyt[:],
                                func=AF.Copy)

                # ---------------- L2 (kt-pair paced by W2 DMAs) ----------
                psl2 = {br: [ps.tile([R, 512], F32, tag="ps",
                                     name=f"psl2_{br}_{n}")
                             for n in range(NCH)] for br in ("m", "s")}
                # W2 streams in column halves: psl2 chunks 0,1 finish a full
                # half-stream early, so their BN2/cat overlap the second half
                xg_late = []
                for half in range(2):
                    for j in range(KT // 2):
                        w = wp.tile([128, 2, C // 2], BF16, tag="w2",
                                    name=f"w2_{half}_{j}")
                        nc.sync.dma_start(
                            out=w[:],
                            in_=w2_p[256 * j:256 * (j + 1),
                                     1024 * half:1024 * (half + 1)].rearrange(
                                "(k p) c -> p k c", p=128))
                        if (half, j) in ((0, 6), (1, 1), (1, 3)):
                            b = 5 + len(xg_late)
                            xg = stream.tile([128, KT, HW], BF16, tag="xg",
                                             name=f"xg{b}")
                            nc.sync.dma_start(
                                out=xg[:],
                                in_=xg_p[b].rearrange("(p j) hw -> p j hw",
                                                      p=128))
                            xg_late.append(xg)
                        for k in range(2):
                            kt = 2 * j + k
                            for br in ("m", "s"):
                                for n in (2 * half, 2 * half + 1):
                                    nc.tensor.matmul(
                                        psl2[br][n][:], y1T[br][:, kt, :],
                                        w[:, k, 512 * (n - 2 * half):
                                          512 * (n - 2 * half + 1)],
                                        start=(kt == 0), stop=(kt == KT - 1))

                # late reduces fill the DVE idle window before BN2
                for i, b in enumerate(range(5, BL)):
                    with nc.allow_low_precision(reason="GAP bf16"):
                        nc.vector.reduce_sum(out=G[:, b, :], in_=xg_late[i][:],
                                             axis=mybir.AxisListType.X)

                # GAP BN(gb) right after the reduces (DVE is idle here)
                nc.vector.tensor_tensor(
                    Gn[:], G[:],
                    sgb[:, None, :].to_broadcast([128, BL, KT]), OP.mult)
                nc.vector.tensor_tensor(
                    Gn[:], Gn[:],
                    tgb[:, None, :].to_broadcast([128, BL, KT]), OP.add)

                # layer-2 BN reps into the shared slots (after BN1 reads;
                # emitted after the W2 DMAs so they can't block the stream)
                for k in ("s", "t"):
                    nc.sync.dma_start(
                        out=srep[(2, k)][:],
                        in_=_raw(scr[(2, k)][:],
                                 [[0, BL], [C, NPARTS], [1, C]]))

                # ---------------- BN2 + relu + outputs --------------
                cat_off = {"m": 3 * C, "s": 3 * C + NPARTS * C}
                bnf_off = {"m": C, "s": 2 * C}
                bnf = {br: stg.tile([BL, C], F32, tag=f"bnf{br}", bufs=1,
                                    name=f"bnf{br}")
                       for br in ("m", "s")}
                x2bs = {}
                catv = {br: out_p[:, cat_off[br]:cat_off[br] + NPARTS * C
                                  ].rearrange("b (q c) -> b q c", c=C)
                        for br in ("m", "s")}
                for ng in range(2):
                    for br in ("m", "s"):
                        for n in (2 * ng, 2 * ng + 1):
                            sl = slice(512 * n, 512 * (n + 1))
                            xs = stg.tile([R, 512], F32, tag="xo",
                                          name=f"xo2{br}{n}")
                            add_eng = nc.vector if br == "m" else nc.gpsimd
                            nc.vector.tensor_tensor(xs[:], psl2[br][n][:],
                                                    srep[(2, "s")][:, sl],
                                                    OP.mult)
                            add_eng.tensor_tensor(xs[:], xs[:],
                                                  srep[(2, "t")][:, sl], OP.add)
                            x2c = stg.tile([R, 512], F32, tag="x2c", bufs=6,
                                           name=f"x2c{br}{n}")
                            nc.scalar.activation(out=x2c[:], in_=xs[:],
                                                 func=AF.Relu)
                            nc.sync.dma_start(out=catv[br][:, :, sl], in_=x2c[:])
                            x2b = stg.tile([R, 512], BF16, tag="x2b", bufs=8,
                                           name=f"x2b{br}{n}")
                            nc.scalar.activation(out=x2b[:], in_=xs[:],
                                                 func=AF.Relu)
                            x2bs[(br, n)] = x2b
                for ng in range(2):
                    for br in ("m", "s"):
                        for n in (2 * ng, 2 * ng + 1):
                            sl = slice(512 * n, 512 * (n + 1))
                            pf = ps.tile([BL, 512], F32, tag="ps",
                                         name=f"pf{br}{n}")
                            nc.tensor.matmul(pf[:], onesblk[:], x2bs[(br, n)][:],
                                             start=True, stop=True)
                            badd = nc.vector if br == "m" else nc.gpsimd
                            nc.vector.tensor_tensor(bnf[br][:, sl], pf[:],
                                                    sgn8[:, sl], OP.mult)
                            badd.tensor_tensor(bnf[br][:, sl], bnf[br][:, sl],
                                               tgn8[:, sl], OP.add)
                for br in ("m", "s"):
                    boff = bnf_off[br]
                    nc.sync.dma_start(out=out_p[:, boff:boff + C],
                                      in_=bnf[br][:])

                # ---------------- GAP out ----------------
                nc.sync.dma_start(
                    out=out_p[:, 0:C].rearrange("b (p j) -> p b j", j=KT),
                    in_=Gn[:])

    legalize_waits(nc)
    return nc


_CACHE = {}


def kernel(_run_kwargs=None, **inputs):
    run_kwargs = _run_kwargs or {}
    if "nc" not in _CACHE:
        _CACHE["nc"] = build_bass()
    nc = _CACHE["nc"]

    B = inputs["x_global"].shape[0]
    n_cores = 8
    bl = B // n_cores
    bf16 = ml_dtypes.bfloat16

    rep_f32 = ["b1", "b2", "g1", "be1", "rm1", "rv1",
               "g2", "be2", "rm2", "rv2",
               "gb_g", "gb_b", "gb_rm", "gb_rv",
               "gn_g", "gn_b", "gn_rm", "gn_rv"]
    w1 = np.ascontiguousarray(inputs["W1"]).astype(bf16)
    w2 = np.ascontiguousarray(inputs["W2"]).astype(bf16)

    in_maps = []
    for c in range(n_cores):
        sl = slice(c * bl, (c + 1) * bl)
        xg = inputs["x_gcn"][sl].reshape(bl, C, HW)
        adj = inputs["adj"][sl]
        bdt = np.zeros((R, R), np.float32)
        for i in range(bl):
            bdt[NPARTS * i:NPARTS * (i + 1), NPARTS * i:NPARTS * (i + 1)] = adj[i].T
        # downsampled mask packed as (p, b, h) with hw = h*128 + p
        mds = inputs["mask"][sl, 0, ::16, ::16].reshape(bl, 2, 128)
        m = {
            "x_gcn_t": np.ascontiguousarray(
                xg.transpose(0, 2, 1)).astype(bf16),
            "x_global": np.ascontiguousarray(
                inputs["x_global"][sl]).reshape(bl, C, HW).astype(bf16),
            "mask_p": np.ascontiguousarray(
                mds.transpose(2, 0, 1)).astype(np.int32),
            "adj_bdt": bdt.astype(bf16),
            "W1": w1,
            "W2": w2,
        }
        for k in rep_f32:
            m[k] = np.ascontiguousarray(inputs[k]).astype(np.float32)
        in_maps.append(m)

    from concourse.bass_utils import run_bass_kernel_spmd
    res = run_bass_kernel_spmd(nc, in_maps, list(range(n_cores)), **run_kwargs)
    out = np.concatenate([res.results[c]["out"] for c in range(n_cores)], axis=0)
    _CACHE["last_results"] = res
    return out


# revision 48
# speedup vs baseline: 1.0383x; 1.0014x over previous
"""Trainium2 Bass kernel for nn_Baseline_SelfGCN (gnn_message_passing).

Data-parallel over batch: 8 NeuronCores x 8 images each.

Final design. All large tensors ship as bfloat16 (halves HBM traffic, the
bottleneck for this memory-regime problem). x_gcn is pre-transposed on the
host to (HW, C); segment pooling runs as one 72-wide seg matmul per
(image, half) against a block-diagonal one-hot, accumulating all images into
four persistent PSUM chunks. x_global streams 0-4 interleaved in phase A
(DVE GAP reduces hide under the stream) and 5-7 inside the W2 stream
(reduces fill the DVE window before BN2). W1/W2 stream as kt-pair DMAs
pacing L1/L2. The midchain folds the count scales into partition-scaled BD
copies (BDm/BDs), applies BN1+relu, then produces the layer-2 lhsT directly
via the transposed bmm y1T = x1_chunk^T @ BD (layer-2 associativity:
BD@(x1@W2) == (BD@x1)@W2), so L2's psl2 chunks are already pre-BN x2 and the
tail is just BN2+relu -> cat DMA + bf16 part-mean matmul + BN(gn) -> bnf DMA.
BN folds run in (128, 144)/(128, 16) layouts (full-partition vector work);
mean factors (1/9, 1/256) fold into the BN scales. Engine placement respects
the hardware rule that GPSIMD cannot touch PSUM: all PSUM-reading elementwise
ops sit on DVE/Act, SBUF-only adds/copies on Pool.

Self-contained: hardcodes shapes; host side only shards/transposes/casts.
"""

import numpy as np
import ml_dtypes

import concourse.bass as bass
import concourse.mybir as mybir
import concourse.tile as tile
from concourse.masks import make_identity

F32 = mybir.dt.float32
BF16 = mybir.dt.bfloat16
I32 = mybir.dt.int32
AF = mybir.ActivationFunctionType
OP = mybir.AluOpType

BL = 8          # images per core
C = 2048
HW = 256        # Hf*Wf
NPARTS = 9      # graph nodes (parts 1..9)
R = BL * NPARTS  # 72 rows = (image, part)
EPS = 1e-5
NCH = 4         # 2048 / 512 N-chunks
KT = 16         # 2048 / 128 K-tiles
PF = 144        # NPARTS*C / 128 fold-layout free size
OUTW = 3 * C + 2 * NPARTS * C  # 43008


def legalize_waits(nc, max_waits=1):
    """Split multi-wait instructions: this walrus build allows only one
    embedded sync-wait per instruction; hoist extras onto standalone
    InstEventSemaphore waits on the same engine."""
    cnt = 0
    for fn in nc.m.functions:
        for blk in fn.blocks:
            out = []
            for inst in blk.instructions:
                si = inst.sync_info
                if si is not None and si.on_wait and len(si.on_wait) > max_waits:
                    waits = list(si.on_wait)
                    for w in waits[:-max_waits]:
                        cnt += 1
                        wi = mybir.InstEventSemaphore(
                            name=f"wsplit{cnt}_{inst.name}", ins=[], outs=[],
                            sync_info=mybir.SyncInfo(on_wait=[w], on_update=[]))
                        wi.engine = inst.engine
                        nc.register_instruction(wi)
                        out.append(wi)
                    si.on_wait = waits[-max_waits:]
                    inst.sync_info = si
                out.append(inst)
            blk.instructions = out
    return cnt


def _raw(ap, dims):
    """Build an AP over the same tensor with explicit [stride, count] dims."""
    return bass.AP(tensor=ap.tensor, offset=ap.offset, ap=dims)


def build_bass():
    nc = bass.Bass()

    xt_p = nc.declare_dram_parameter("x_gcn_t", [BL, HW, C], BF16, isOutput=False)
    xg_p = nc.declare_dram_parameter("x_global", [BL, C, HW], BF16, isOutput=False)
    mk_p = nc.declare_dram_parameter("mask_p", [128, BL, 2], I32, isOutput=False)
    bd_p = nc.declare_dram_parameter("adj_bdt", [R, R], BF16, isOutput=False)
    w1_p = nc.declare_dram_parameter("W1", [C, C], BF16, isOutput=False)
    w2_p = nc.declare_dram_parameter("W2", [C, C], BF16, isOutput=False)
    b1_p = nc.declare_dram_parameter("b1", [C], F32, isOutput=False)
    b2_p = nc.declare_dram_parameter("b2", [C], F32, isOutput=False)
    bn1 = {k: nc.declare_dram_parameter(k + "1", [NPARTS * C], F32, isOutput=False)
           for k in ("g", "be", "rm", "rv")}
    bn2 = {k: nc.declare_dram_parameter(k + "2", [NPARTS * C], F32, isOutput=False)
           for k in ("g", "be", "rm", "rv")}
    gb = {k: nc.declare_dram_parameter("gb_" + k, [C], F32, isOutput=False)
          for k in ("g", "b", "rm", "rv")}
    gn = {k: nc.declare_dram_parameter("gn_" + k, [C], F32, isOutput=False)
          for k in ("g", "b", "rm", "rv")}
    out_p = nc.declare_dram_parameter("out", [BL, OUTW], F32, isOutput=True)

    with tile.TileContext(nc) as tc:
        with (
            tc.tile_pool(name="consts", bufs=1) as cs,
            tc.tile_pool(name="ps", bufs=8, space="PSUM") as ps,
            tc.tile_pool(name="dram", bufs=1, space="DRAM") as dp,
        ):
            # ---------------- persistent tiles ----------------
            BD = cs.tile([R, R], BF16)
            BDm = cs.tile([R, R], BF16)   # BD partition-scaled by 1/count
            BDs = cs.tile([R, R], BF16)   # BD partition-scaled by keep/count
            mr = cs.tile([128, BL, 2], I32)
            mrf = cs.tile([128, BL, 2], F32)
            # block-diagonal one-hot: oh[:, b, h, 9b:9b+9] is image b's
            # one-hot, other columns zero -> all-image seg pooling uses it
            # as a 72-wide stationary with per-(b,h) k-tiles
            oh = cs.tile([128, BL, 2, R], BF16)
            mfT = cs.tile([128, KT, R], BF16)
            G = cs.tile([128, BL, KT], BF16)
            Gn = cs.tile([128, BL, KT], F32)
            selfm98 = cs.tile([NPARTS, BL], F32)
            rec98 = cs.tile([NPARTS, BL], F32)
            mrec72 = cs.tile([R, 1], F32)
            srec72 = cs.tile([R, 1], F32)
            sgb = cs.tile([128, KT], F32)   # c = p*16 + j (matches G layout)
            tgb = cs.tile([128, KT], F32)
            sgn16 = cs.tile([128, KT], F32)
            tgn16 = cs.tile([128, KT], F32)
            sgn8 = cs.tile([BL, C], BF16)
            tgn8 = cs.tile([BL, C], BF16)
            # layer-1 and layer-2 BN reps share slots (srep2 loads after BN1)
            srep = {}
            for li in (1, 2):
                for k in ("s", "t"):
                    srep[(li, k)] = cs.tile([R, C], BF16, tag=f"rep_{k}",
                                            name=f"rep_{k}{li}")

            # DRAM scratch
            scr = {(li, k): dp.tile([NPARTS, C], BF16, name=f"scr_{k}{li}")
                   for li in (1, 2) for k in ("s", "t")}
            scr_b = {1: dp.tile([NPARTS, C], F32, name="scr_b1"),
                     2: dp.tile([NPARTS, C], F32, name="scr_b2")}
            scr_gn = {k: dp.tile([C], BF16, name=f"scr_gn{k}") for k in ("s", "t")}
            scr_sm = dp.tile([NPARTS, BL], F32, name="scr_sm")
            scr_rc = dp.tile([NPARTS, BL], F32, name="scr_rc")

            # mask first: unblocks onehot compute immediately
            nc.sync.dma_start(out=mr[:], in_=mk_p[:])

            # ---------------- constants (gpsimd, no DMA) ----------------
            ident = cs.tile([128, 128], BF16)
            make_identity(nc, ident[:])

            iota_i = cs.tile([128, NPARTS], I32)
            nc.gpsimd.iota(iota_i[:], pattern=[[1, NPARTS]], base=1, channel_multiplier=0)
            iota_f = cs.tile([128, NPARTS], F32)
            nc.gpsimd.tensor_copy(out=iota_f[:], in_=iota_i[:])

            ones_col = cs.tile([128, 1], BF16)
            nc.gpsimd.memset(ones_col[:], 1.0)

            # strictly-lower-triangular L9: L[q,p] = 1 if q < p
            L9 = cs.tile([NPARTS, NPARTS], BF16)
            nc.gpsimd.memset(L9[:], 0.0)
            nc.gpsimd.affine_select(
                out=L9[:], in_=L9[:], compare_op=OP.is_ge, fill=1.0,
                base=0, pattern=[[-1, NPARTS]], channel_multiplier=1,
            )

            # block "mean over parts" matrix (72, 8): ones on image blocks
            # (the 1/NPARTS is folded into the BN(gn) scale)
            onesblk = cs.tile([R, BL], BF16)
            nc.gpsimd.memset(onesblk[:], 1.0)
            nc.gpsimd.affine_select(
                out=onesblk[:], in_=onesblk[:], compare_op=OP.is_ge, fill=0.0,
                base=0, pattern=[[-NPARTS, BL]], channel_multiplier=1)
            nc.gpsimd.affine_select(
                out=onesblk[:], in_=onesblk[:], compare_op=OP.is_ge, fill=0.0,
                base=NPARTS - 1, pattern=[[NPARTS, BL]], channel_multiplier=-1)

            # one-hot compare ops (DVE, first in its queue)
            nc.gpsimd.memset(oh[:], 0.0)
            nc.vector.tensor_copy(out=mrf[:], in_=mr[:])
            for b in range(BL):
                for h in range(2):
                    nc.vector.tensor_scalar(
                        out=oh[:, b, h, NPARTS * b:NPARTS * (b + 1)],
                        in0=iota_f[:],
                        scalar1=mrf[:, b, h:h + 1], scalar2=None, op0=OP.is_equal)

            with (
                tc.tile_pool(name="small", bufs=2) as sp,
                tc.tile_pool(name="stream", bufs=3) as stream,
                tc.tile_pool(name="foldtmp", bufs=1) as ftp,
                tc.tile_pool(name="wp", bufs=4) as wp,
                tc.tile_pool(name="mm", bufs=1) as mm,
                tc.tile_pool(name="stage", bufs=3) as stg,
            ):
                # fold tiles (computed during phase A)
                fold = {}
                for li in (1, 2):
                    for k in ("st", "gt", "tt", "rmt", "bet", "bt"):
                        fold[(li, k)] = ftp.tile([128, PF], F32, tag=f"f_{k}",
                                                 name=f"f_{k}{li}")
                    for k in ("sb", "tb"):
                        fold[(li, k)] = ftp.tile([128, PF], BF16, tag=f"f_{k}",
                                                 name=f"f_{k}{li}")
                gf = {}
                for pr in ("gb", "gn"):
                    for k in ("g", "b", "rm", "rv"):
                        gf[(pr, k)] = ftp.tile([128, KT], F32, tag=f"gf_{k}",
                                               name=f"gf_{pr}{k}")
                gnb = {k: ftp.tile([128, KT], BF16, tag=f"gnb_{k}",
                                   name=f"gnb{k}") for k in ("s", "t")}

                def emit_small_dmas(step):
                    """Tiny loads/bounces interleaved between stream DMAs."""
                    if step == 0:
                        nc.sync.dma_start(out=BD[:], in_=bd_p[:])
                        nc.sync.dma_start(
                            out=scr_b[1][:],
                            in_=b1_p[None, :].to_broadcast([NPARTS, C]))
                        nc.sync.dma_start(
                            out=scr_b[2][:],
                            in_=b2_p[None, :].to_broadcast([NPARTS, C]))
                    elif step == 1:
                        for li, bnp in ((1, bn1), (2, bn2)):
                            for k, src in (("st", bnp["rv"]), ("gt", bnp["g"]),
                                           ("rmt", bnp["rm"]), ("bet", bnp["be"])):
                                nc.sync.dma_start(
                                    out=fold[(li, k)][:],
                                    in_=src.rearrange("(p f) -> p f", f=PF))
                            nc.sync.dma_start(
                                out=fold[(li, "bt")][:],
                                in_=_raw(scr_b[li][:], [[PF, 128], [1, PF]]))
                        for pr, d in (("gb", gb), ("gn", gn)):
                            for k in ("g", "b", "rm", "rv"):
                                nc.sync.dma_start(
                                    out=gf[(pr, k)][:],
                                    in_=d[k].rearrange("(p j) -> p j", j=KT))
                    elif step == 2:
                        # fold compute done by now; write scr
                        for li in (1, 2):
                            for k, t in (("s", "sb"), ("t", "tb")):
                                nc.sync.dma_start(
                                    out=_raw(scr[(li, k)][:],
                                             [[PF, 128], [1, PF]]),
                                    in_=fold[(li, t)][:])
                        nc.sync.dma_start(
                            out=scr_gn["s"][:].rearrange("(p j) -> p j", j=KT),
                            in_=gnb["s"][:])
                        nc.sync.dma_start(
                            out=scr_gn["t"][:].rearrange("(p j) -> p j", j=KT),
                            in_=gnb["t"][:])
                    elif step == 3:
                        nc.sync.dma_start(
                            out=sgn8[:],
                            in_=_raw(scr_gn["s"][:], [[0, BL], [1, C]]))
                        nc.sync.dma_start(
                            out=tgn8[:],
                            in_=_raw(scr_gn["t"][:], [[0, BL], [1, C]]))
                        # layer-1 BN reps (scr written at step 2)
                        for k in ("s", "t"):
                            nc.sync.dma_start(
                                out=srep[(1, k)][:],
                                in_=_raw(scr[(1, k)][:],
                                         [[0, BL], [C, NPARTS], [1, C]]))

                def emit_folds():
                    """BN fold arithmetic in (128, PF) / (128, KT) layouts."""
                    for li in (1, 2):
                        st, gt, tt = (fold[(li, "st")], fold[(li, "gt")],
                                      fold[(li, "tt")])
                        rmt, bet, bt = (fold[(li, "rmt")], fold[(li, "bet")],
                                        fold[(li, "bt")])
                        nc.vector.tensor_scalar_add(st[:], st[:], EPS)
                        nc.scalar.activation(out=st[:], in_=st[:], func=AF.Sqrt)
                        nc.vector.reciprocal(out=st[:], in_=st[:])
                        nc.vector.tensor_mul(st[:], st[:], gt[:])
                        nc.vector.tensor_sub(tt[:], bt[:], rmt[:])
                        nc.vector.tensor_mul(tt[:], tt[:], st[:])
                        nc.vector.tensor_add(tt[:], tt[:], bet[:])
                        nc.vector.tensor_copy(out=fold[(li, "sb")][:], in_=st[:])
                        nc.vector.tensor_copy(out=fold[(li, "tb")][:], in_=tt[:])
                    for pr, dst_s, dst_t, inv in (("gb", sgb, tgb, 1.0 / HW),
                                                  ("gn", sgn16, tgn16,
                                                   1.0 / NPARTS)):
                        nc.vector.tensor_scalar_add(dst_s[:], gf[(pr, "rv")][:],
                                                    EPS)
                        nc.scalar.activation(out=dst_s[:], in_=dst_s[:],
                                             func=AF.Sqrt)
                        nc.vector.reciprocal(out=dst_s[:], in_=dst_s[:])
                        nc.vector.tensor_mul(dst_s[:], dst_s[:],
                                             gf[(pr, "g")][:])
                        nc.vector.tensor_mul(dst_t[:], gf[(pr, "rm")][:],
                                             dst_s[:])
                        nc.vector.tensor_sub(dst_t[:], gf[(pr, "b")][:],
                                             dst_t[:])
                        # fold mean normalization into the scale
                        nc.vector.tensor_scalar_mul(dst_s[:], dst_s[:], inv)
                    nc.vector.tensor_copy(out=gnb["s"][:], in_=sgn16[:])
                    nc.vector.tensor_copy(out=gnb["t"][:], in_=tgn16[:])

                # ------------- phase A: x streams + pooling + GAP ----------
                # all-image seg pooling: mfeat (72, 2048) accumulates over
                # 16 (image, half) k-tiles with the block-diag one-hot
                psm = [ps.tile([R, 512], F32, tag="ps", name=f"psm{n}")
                       for n in range(NCH)]
                for b in range(BL):
                    xt = stream.tile([128, 2, C], BF16, tag="xt", name=f"xt{b}")
                    nc.sync.dma_start(
                        out=xt[:],
                        in_=xt_p[b].rearrange("(h p) c -> p h c", p=128))
                    if b < 5:
                        xg = stream.tile([128, KT, HW], BF16, tag="xg",
                                         name=f"xg{b}")
                        nc.sync.dma_start(
                            out=xg[:],
                            in_=xg_p[b].rearrange("(p j) hw -> p j hw", p=128))
                    if b == 0:
                        # loads first, then the fold arithmetic that uses them
                        emit_small_dmas(0)
                        emit_small_dmas(1)
                        emit_folds()
                    elif b in (2, 4):
                        emit_small_dmas(b // 2 + 1)
                    for h in range(2):
                        for n in range(NCH):
                            nc.tensor.matmul(
                                psm[n][:], oh[:, b, h, :],
                                xt[:, h, 512 * n:512 * (n + 1)],
                                start=(b == 0 and h == 0),
                                stop=(b == BL - 1 and h == 1))
                    if b < 5:
                        with nc.allow_low_precision(reason="GAP bf16"):
                            nc.vector.reduce_sum(out=G[:, b, :], in_=xg[:],
                                                 axis=mybir.AxisListType.X)

                # counts/drop logic (Pool + PE; runs behind the stream)
                for b in range(BL):
                    pcnt = ps.tile([NPARTS, 1], F32, tag="ps", name=f"pcnt{b}")
                    for h in range(2):
                        nc.tensor.matmul(pcnt[:],
                                         oh[:, b, h, NPARTS * b:NPARTS * (b + 1)],
                                         ones_col[:],
                                         start=(h == 0), stop=(h == 1))
                    nc.vector.tensor_scalar_add(rec98[:, b:b + 1], pcnt[:], 1e-8)
                    nc.vector.reciprocal(out=rec98[:, b:b + 1],
                                         in_=rec98[:, b:b + 1])
                    pres = sp.tile([NPARTS, 1], BF16, tag="pres", name=f"pres{b}")
                    nc.vector.tensor_scalar(out=pres[:], in0=pcnt[:], scalar1=0.0,
                                            scalar2=None, op0=OP.is_gt)
                    ppre = ps.tile([NPARTS, 1], F32, tag="ps", name=f"ppre{b}")
                    nc.tensor.matmul(ppre[:], L9[:], pres[:], start=True, stop=True)
                    isz = sp.tile([NPARTS, 1], F32, tag="isz", name=f"isz{b}")
                    nc.vector.tensor_scalar(out=isz[:], in0=ppre[:], scalar1=0.0,
                                            scalar2=None, op0=OP.is_equal)
                    nc.gpsimd.tensor_mul(isz[:], isz[:], pres[:])
                    nc.gpsimd.tensor_scalar(out=selfm98[:, b:b + 1], in0=isz[:],
                                            scalar1=-1.0, scalar2=1.0,
                                            op0=OP.mult, op1=OP.add)

                # mfeat (psum) -> SBUF bf16 -> transpose into mfT (grouped)
                mfsb = mm.tile([R, C], BF16, tag="mfsb")
                for n in range(NCH):
                    sl = slice(512 * n, 512 * (n + 1))
                    if n % 2 == 0:
                        nc.scalar.activation(out=mfsb[:, sl], in_=psm[n][:],
                                             func=AF.Copy)
                    else:
                        nc.vector.tensor_copy(out=mfsb[:, sl], in_=psm[n][:])
                for g4 in range(4):
                    pmt = ps.tile([128, 4, R], BF16, tag="ps", name=f"pmt{g4}")
                    for k in range(4):
                        kt = 4 * g4 + k
                        nc.tensor.transpose(pmt[:, k, :],
                                            mfsb[:, 128 * kt:128 * (kt + 1)],
                                            ident[0:R, 0:R])
                    eng = (nc.vector, nc.scalar, nc.vector, nc.scalar)[g4]
                    if eng is nc.scalar:
                        eng.activation(out=mfT[:, 4 * g4:4 * g4 + 4, :],
                                       in_=pmt[:], func=AF.Copy)
                    else:
                        eng.tensor_copy(out=mfT[:, 4 * g4:4 * g4 + 4, :],
                                        in_=pmt[:])

                # ---- L1 n-outer (W1 resident) + lagged per-chunk midchain ---
                s_raw = mm.tile([R, C], BF16, tag="sraw")
                x1 = {br: mm.tile([R, C], BF16, tag=f"x1{br}", name=f"x1{br}")
                      for br in ("m", "s")}
                y1T = {br: mm.tile([128, KT, R], BF16, tag=f"y1T{br}",
                                   name=f"y1T{br}")
                       for br in ("m", "s")}

                psl1 = [ps.tile([R, 512], F32, tag="ps", name=f"psl1_{n}")
                        for n in range(NCH)]
                for j in range(KT // 2):
                    w = wp.tile([128, 2, C], BF16, tag="w", name=f"w1_{j}")
                    nc.sync.dma_start(
                        out=w[:],
                        in_=w1_p[256 * j:256 * (j + 1), :].rearrange(
                            "(k p) c -> p k c", p=128))
                    if j == 1:
                        # counts done by now (Pool/PE ran behind the stream)
                        nc.sync.dma_start(out=scr_sm[:], in_=selfm98[:])
                        nc.sync.dma_start(out=scr_rc[:], in_=rec98[:])
                    if j == 3:
                        nc.sync.dma_start(
                            out=mrec72[:],
                            in_=_raw(scr_rc[:], [[1, BL], [BL, NPARTS], [0, 1]]))
                        nc.sync.dma_start(
                            out=srec72[:],
                            in_=_raw(scr_sm[:], [[1, BL], [BL, NPARTS], [0, 1]]))
                        nc.gpsimd.tensor_mul(srec72[:], srec72[:], mrec72[:])
                        # count scales fold into partition-scaled BDs
                        nc.gpsimd.tensor_scalar(
                            out=BDm[:], in0=BD[:], scalar1=mrec72[:, 0:1],
                            scalar2=None, op0=OP.mult)
                        nc.gpsimd.tensor_scalar(
                            out=BDs[:], in0=BD[:], scalar1=srec72[:, 0:1],
                            scalar2=None, op0=OP.mult)
                    for k in range(2):
                        kt = 2 * j + k
                        for n in range(NCH):
                            nc.tensor.matmul(psl1[n][:], mfT[:, kt, :],
                                             w[:, k, 512 * n:512 * (n + 1)],
                                             start=(kt == 0), stop=(kt == KT - 1))

                # stage-major midchain: copies, all po1, BN1+relu, y1T bmms
                for n in range(NCH):
                    sl = slice(512 * n, 512 * (n + 1))
                    cp = (nc.scalar, nc.vector, nc.scalar, nc.vector)[n]
                    if cp is nc.scalar:
                        cp.activation(out=s_raw[:, sl], in_=psl1[n][:],
                                      func=AF.Copy)
                    else:
                        cp.tensor_copy(out=s_raw[:, sl], in_=psl1[n][:])
                po1 = {}
                for br, bd in (("m", BDm), ("s", BDs)):
                    for n in range(NCH):
                        po = ps.tile([R, 512], F32, tag="ps", name=f"po1{br}{n}")
                        nc.tensor.matmul(po[:], bd[:],
                                         s_raw[:, 512 * n:512 * (n + 1)],
                                         start=True, stop=True)
                        po1[(br, n)] = po
                for br in ("m", "s"):
                    add_eng = nc.vector if br == "m" else nc.gpsimd
                    for n in range(NCH):
                        sl = slice(512 * n, 512 * (n + 1))
                        xs = stg.tile([R, 512], F32, tag="xo", name=f"xo1{br}{n}")
                        nc.vector.tensor_tensor(xs[:], po1[(br, n)][:],
                                                srep[(1, "s")][:, sl], OP.mult)
                        add_eng.tensor_tensor(xs[:], xs[:],
                                              srep[(1, "t")][:, sl], OP.add)
                        nc.scalar.activation(out=x1[br][:, sl], in_=xs[:],
                                             func=AF.Relu)
                # transposed bmm: y1T chunks = x1_chunk^T @ BD directly
                for br in ("m", "s"):
                    for n in range(NCH):
                        pyt = ps.tile([128, 4, R], F32, tag="ps",
                                      name=f"pyt{br}{n}")
                        for k in range(4):
                            kt = 4 * n + k
                            nc.tensor.matmul(pyt[:, k, :],
                                             x1[br][:, 128 * kt:128 * (kt + 1)],
                                             BD[:], start=True, stop=True)
                        if br == "m":
                            nc.vector.tensor_copy(
                                out=y1T[br][:, 4 * n:4 * n + 4, :], in_=pyt[:])
                        else:
                            nc.scalar.activation(
                                out=y1T[br][:, 4 * n:4 * n + 4, :], in_=pyt[:],
                                func=AF.Copy)

                # ---------------- L2 (kt-pair paced by W2 DMAs) ----------
                psl2 = {br: [ps.tile([R, 512], F32, tag="ps",
                                     name=f"psl2_{br}_{n}")
                             for n in range(NCH)] for br in ("m", "s")}
                # W2 streams in column halves: psl2 chunks 0,1 finish a full
                # half-stream early, so their BN2/cat overlap the second half
                xg_late = []
                for half in range(2):
                    for j in range(KT // 2):
                        w = wp.tile([128, 2, C // 2], BF16, tag="w2",
                                    name=f"w2_{half}_{j}")
                        nc.sync.dma_start(
                            out=w[:],
                            in_=w2_p[256 * j:256 * (j + 1),
                                     1024 * half:1024 * (half + 1)].rearrange(
                                "(k p) c -> p k c", p=128))
                        if (half, j) in ((0, 6), (1, 1), (1, 3)):
                            b = 5 + len(xg_late)
                            xg = stream.tile([128, KT, HW], BF16, tag="xg",
                                             name=f"xg{b}")
                            nc.sync.dma_start(
                                out=xg[:],
                                in_=xg_p[b].rearrange("(p j) hw -> p j hw",
                                                      p=128))
                            xg_late.append(xg)
                        for k in range(2):
                            kt = 2 * j + k
                            for br in ("m", "s"):
                                for n in (2 * half, 2 * half + 1):
                                    nc.tensor.matmul(
                                        psl2[br][n][:], y1T[br][:, kt, :],
                                        w[:, k, 512 * (n - 2 * half):
                                          512 * (n - 2 * half + 1)],
                                        start=(kt == 0), stop=(kt == KT - 1))

                # late reduces fill the DVE idle window before BN2
                for i, b in enumerate(range(5, BL)):
                    with nc.allow_low_precision(reason="GAP bf16"):
                        nc.vector.reduce_sum(out=G[:, b, :], in_=xg_late[i][:],
                                             axis=mybir.AxisListType.X)

                # GAP BN(gb) right after the reduces (DVE is idle here)
                nc.vector.tensor_tensor(
                    Gn[:], G[:],
                    sgb[:, None, :].to_broadcast([128, BL, KT]), OP.mult)
                nc.vector.tensor_tensor(
                    Gn[:], Gn[:],
                    tgb[:, None, :].to_broadcast([128, BL, KT]), OP.add)

                # layer-2 BN reps into the shared slots (after BN1 reads;
                # emitted after the W2 DMAs so they can't block the stream)
                for k in ("s", "t"):
                    nc.sync.dma_start(
                        out=srep[(2, k)][:],
                        in_=_raw(scr[(2, k)][:],
                                 [[0, BL], [C, NPARTS], [1, C]]))

                # ---------------- BN2 + relu + outputs --------------
                cat_off = {"m": 3 * C, "s": 3 * C + NPARTS * C}
                bnf_off = {"m": C, "s": 2 * C}
                bnf = {br: stg.tile([BL, C], F32, tag=f"bnf{br}", bufs=1,
                                    name=f"bnf{br}")
                       for br in ("m", "s")}
                x2bs = {}
                catv = {br: out_p[:, cat_off[br]:cat_off[br] + NPARTS * C
                                  ].rearrange("b (q c) -> b q c", c=C)
                        for br in ("m", "s")}
                for ng in range(2):
                    for br in ("m", "s"):
                        for n in (2 * ng, 2 * ng + 1):
                            sl = slice(512 * n, 512 * (n + 1))
                            xs = stg.tile([R, 512], F32, tag="xo",
                                          name=f"xo2{br}{n}")
                            add_eng = nc.vector if br == "m" else nc.gpsimd
                            nc.vector.tensor_tensor(xs[:], psl2[br][n][:],
                                                    srep[(2, "s")][:, sl],
                                                    OP.mult)
                            add_eng.tensor_tensor(xs[:], xs[:],
                                                  srep[(2, "t")][:, sl], OP.add)
                            x2c = stg.tile([R, 512], F32, tag="x2c", bufs=6,
                                           name=f"x2c{br}{n}")
                            nc.scalar.activation(out=x2c[:], in_=xs[:],
                                                 func=AF.Relu)
                            nc.sync.dma_start(out=catv[br][:, :, sl], in_=x2c[:])
                            x2b = stg.tile([R, 512], BF16, tag="x2b", bufs=8,
                                           name=f"x2b{br}{n}")
                            nc.scalar.activation(out=x2b[:], in_=xs[:],
                                                 func=AF.Relu)
                            x2bs[(br, n)] = x2b
                for ng in range(2):
                    for br in ("m", "s"):
                        for n in (2 * ng, 2 * ng + 1):
                            sl = slice(512 * n, 512 * (n + 1))
                            pf = ps.tile([BL, 512], F32, tag="ps",
                                         name=f"pf{br}{n}")
                            nc.tensor.matmul(pf[:], onesblk[:], x2bs[(br, n)][:],
                                             start=True, stop=True)
                            badd = nc.vector if br == "m" else nc.gpsimd
                            nc.vector.tensor_tensor(bnf[br][:, sl], pf[:],
                                                    sgn8[:, sl], OP.mult)
                            badd.tensor_tensor(bnf[br][:, sl], bnf[br][:, sl],
                                               tgn8[:, sl], OP.add)
                        # drain this branch's finished half immediately
                        boff = bnf_off[br] + 1024 * ng
                        nc.sync.dma_start(
                            out=out_p[:, boff:boff + 1024],
                            in_=bnf[br][:, 1024 * ng:1024 * (ng + 1)])

                # ---------------- GAP out ----------------
                nc.sync.dma_start(
                    out=out_p[:, 0:C].rearrange("b (p j) -> p b j", j=KT),
                    in_=Gn[:])

    legalize_waits(nc)
    return nc


_CACHE = {}


def kernel(_run_kwargs=None, **inputs):
    run_kwargs = _run_kwargs or {}
    if "nc" not in _CACHE:
        _CACHE["nc"] = build_bass()
    nc = _CACHE["nc"]

    B = inputs["x_global"].shape[0]
    n_cores = 8
    bl = B // n_cores
    bf16 = ml_dtypes.bfloat16

    rep_f32 = ["b1", "b2", "g1", "be1", "rm1", "rv1",
               "g2", "be2", "rm2", "rv2",
               "gb_g", "gb_b", "gb_rm", "gb_rv",
               "gn_g", "gn_b", "gn_rm", "gn_rv"]
    w1 = np.ascontiguousarray(inputs["W1"]).astype(bf16)
    w2 = np.ascontiguousarray(inputs["W2"]).astype(bf16)

    in_maps = []
    for c in range(n_cores):
        sl = slice(c * bl, (c + 1) * bl)
        xg = inputs["x_gcn"][sl].reshape(bl, C, HW)
        adj = inputs["adj"][sl]
        bdt = np.zeros((R, R), np.float32)
        for i in range(bl):
            bdt[NPARTS * i:NPARTS * (i + 1), NPARTS * i:NPARTS * (i + 1)] = adj[i].T
        # downsampled mask packed as (p, b, h) with hw = h*128 + p
        mds = inputs["mask"][sl, 0, ::16, ::16].reshape(bl, 2, 128)
        m = {
            "x_gcn_t": np.ascontiguousarray(
                xg.transpose(0, 2, 1)).astype(bf16),
            "x_global": np.ascontiguousarray(
                inputs["x_global"][sl]).reshape(bl, C, HW).astype(bf16),
            "mask_p": np.ascontiguousarray(
                mds.transpose(2, 0, 1)).astype(np.int32),
            "adj_bdt": bdt.astype(bf16),
            "W1": w1,
            "W2": w2,
        }
        for k in rep_f32:
            m[k] = np.ascontiguousarray(inputs[k]).astype(np.float32)
        in_maps.append(m)

    from concourse.bass_utils import run_bass_kernel_spmd
    res = run_bass_kernel_spmd(nc, in_maps, list(range(n_cores)), **run_kwargs)
    out = np.concatenate([res.results[c]["out"] for c in range(n_cores)], axis=0)
    _CACHE["last_results"] = res
    return out
